# revision 60
# baseline (speedup 1.0000x reference)
"""Trainium2 Bass kernel for nn_CLFBlock (linear -> LIF scan -> linear -> T-mean -> log_softmax).

Self-contained: hardcodes shapes T=32, B=512, D=1024, C=1000 and data-parallel
sharding of the batch dim across 8 NeuronCores.

Math notes:
  h = x @ W1.T + b1                      (fp8 on the PE, fp32 accum)
  LIF (tau=2, v_th=1, hard reset to 0):
     v' = 0.5*v + 0.5*h
     s  = (v' >= 1);  v = v' * (v' < 1)
  Scan state is kept pre-halved:  hh = 0.5*h + 0.5*b1, and per step one fused
  DVE op advances the pre-reset voltage w:
     w_new = select(w_old < 1, w_old, 0) * 0.5 + hh      (VectorE, serial)
  All 33 w states live in SBUF (one [128,512] w0 tile + 16 [128,1024] pair
  tiles) so spike masks can be produced off the scan's critical chain:
    t 0..15  : m = (w<1) inline on VectorE (bf16, 2x mode), identity -1
               (per-step contribution s_t - 1) -> msumA
    t 16..23 : m = sign(w-1) as four 1024-wide pair ops on ScalarE,
               identity +0.5 (contribution s_t - 0.5) -> msumB
    t 24..31 : four 1024-wide (w<1) pair ops on VectorE after the scan,
               identity -1 -> msumB
  (pair tiles keep the cross-engine mask reads whole-tile; the tile dep
  tracker mishandles sub-range reads of one big tile, and emission order
  defines write->read deps, so the sign pairs are emitted inside the scan
  loop right after the LIF step that completes their pair.)
  ssumA = msumA + 16 in [0,16], ssumB = msumB + 12 in [0,16]: both integer,
  exact in fp8e4m3; the identity-sum spike counts feed two fp8 DoubleRow
  mm2 passes into one PSUM pair, with b2*T*PRESCALE added via a ones-row
  matmul.  out = log_softmax(psy / (T*PRESCALE), axis=1): no max-shift
  needed (|y| small), Exp's free-dim accumulator replaces the reduce, and
  a single ACT table set covers Identity/Copy/Exp/Ln/Sign so no mid-kernel
  table reloads occur.

Layout: mm1 contraction d sits on partitions as d = p*8 + dj so every load
descriptor is 4-12KB contiguous. mm1 output keeps e = j*128 + p, so W2/ssum
keep the (ej p) layout. x is split into tb chunks of 8,8,8,4,4 timesteps;
the small tail groups land the last h earlier so the scan tail isn't gated
by the full mm1. Loads split across the two HW-DGE rings: x chunk0 leads
the scalar ring (it gates mm1 g0), W1 leads the sync ring; b1 comes in as
[8,128] (8 big descriptors instead of 128 tiny ones, ~8us faster) and is
transposed on the PE via an identity matmul into a spare msumA corner.
A-half spike-sum identity matmuls interleave into the mm1 emission so the
PE queue tail stays short, and paced junk matmuls bridge the PE idle gaps
(warm-up until x0 lands, and the mask-production gaps before mm2) so the
activity-gated PE clock does not droop -- idle-then-resume runs matmuls at
~2x the warm latency until the clock ramps again.
"""

import numpy as np
from contextlib import ExitStack

import concourse.bass as bass
import concourse.tile as tile
from concourse import bacc, mybir
from concourse.bass_utils import run_bass_kernel_spmd

N_CORES = 8
N_WARM = 12              # PE warm-up dummy matmuls (full array, 512 cols)


def _lif_op():
    """Fused LIF step as a custom DVE op:
         out = select(in0 < s0, in0, 0) * s1 + in1
       i.e. w_new = reset(w_old)*0.5 + hh  in a single VectorE instruction."""
    from concourse import dve_ops
    from concourse.dve_spec import Spec, Src0, Src1, Zero, C0, C1, select, lower
    from concourse.dve_uop import DveOpSpec

    for op in dve_ops.OPS:
        if op.name == "LIF_STEP_ANT":
            return op
    spec = Spec(
        body=select(Src0 < C0, Src0, Zero) * C1 + Src1,
        reference=lambda in0, in1, s0, s1, imm2: (
            np.where(in0.astype(np.float32) < s0, in0.astype(np.float32), 0.0) * s1
            + in1.astype(np.float32)).astype(np.float32),
    )
    row = dve_ops._CUSTOM_DVE_ROW_BASE + len(dve_ops.OPS)
    shas = {}
    for ver in ("v3", "v4"):
        try:
            shas[ver] = DveOpSpec(name="LIF_STEP_ANT", opcode=row,
                                  uops=lower(spec, ver=ver), rd1_en=True).sha(ver)
        except Exception:
            pass
    op = dve_ops.DveOp("LIF_STEP_ANT", spec, subdim=False, uops_sha=shas)
    dve_ops.OPS.append(op)
    dve_ops._SUB_OPCODE_FOR_NAME[op.name] = row
    dve_ops.CUSTOM_DVE_SPECS[op.name] = spec
    return op


T, B, D, C = 32, 512, 1024, 1000
BC = B // N_CORES          # 64 rows per core
TB = T * BC                # 2048 matmul rows per core
# mm1 tb chunks (t0, nt): small tail groups land the last h early
CHUNKS = [(0, 8), (8, 8), (16, 8), (24, 4), (28, 4)]
CH_OFF = [0, 4096, 8192, 12288, 14336]   # free-dim offsets (nt*512 elems)
FP32 = mybir.dt.float32
BF16 = mybir.dt.bfloat16
FP8 = mybir.dt.float8e4
W1_PRESCALE = 256.0   # host multiplies W1/W2 by this (exact power of 2) so
                      # their small uniform(-1/32,1/32) values stay in
                      # fp8e4m3's normal range; compensated on readout
AF = mybir.ActivationFunctionType
OP = mybir.AluOpType


def _prefer_combined_act_table(arch: str):
    """Force every activation we use (Identity/Copy/Exp/Ln/Sign) to resolve
    to the single set containing them all -> zero mid-kernel table reloads."""
    from concourse.hw_specs import get_activation_tables
    t = get_activation_tables(arch)
    target = "natural_log_exp_and_others"
    if target not in t:
        return
    for k, v in t.items():
        if k != target:
            v.clear()


def build_program():
    nc = bacc.Bacc("TRN2", target_bir_lowering=False, debug=False, num_devices=N_CORES)
    try:
        _prefer_combined_act_table(nc.m.arch)
    except Exception:
        pass

    xt_d = nc.dram_tensor("xT", [128, 16384], FP8, kind="ExternalInput").ap()
    w1t_d = nc.dram_tensor("W1T", [128, 8 * 1024], FP8, kind="ExternalInput").ap()
    b1_d = nc.dram_tensor("b1", [8, 128], FP32, kind="ExternalInput").ap()
    w2t_d = nc.dram_tensor("W2T", [D, C], FP8, kind="ExternalInput").ap()
    b2_d = nc.dram_tensor("b2", [C], FP32, kind="ExternalInput").ap()
    y_d = nc.dram_tensor("y", [BC, C], FP32, kind="ExternalOutput").ap()

    with tile.TileContext(nc) as tc, ExitStack() as ctx:
        persist = ctx.enter_context(tc.tile_pool(name="persist", bufs=1))
        mpool = ctx.enter_context(tc.tile_pool(name="mpool", bufs=16))
        small = ctx.enter_context(tc.tile_pool(name="small", bufs=1))
        ps_h = ctx.enter_context(tc.tile_pool(name="ps_h", bufs=4, space="PSUM"))
        ps_ms = ctx.enter_context(tc.tile_pool(name="ps_ms", bufs=1, space="PSUM"))
        ps_y = ctx.enter_context(tc.tile_pool(name="ps_y", bufs=1, space="PSUM"))

        # ---- x0 + b1/b2 on the scalar ring (x0 first: it gates g0). b1
        # comes in as [8, 128] (8 big descriptors) and is PE-transposed.
        b1_sb = small.tile([8, 128], FP32)
        b2_sb = small.tile([1, C], FP32)

        # ---- big loads spread over four rings; priority order per ring ----
        w1t = persist.tile([128, 8 * 1024], FP8)
        xt = persist.tile([128, 16384], FP8)
        w2t = persist.tile([128, 8 * 1024], FP8)
        w1t3 = w1t[:].rearrange("p (j dj e) -> p j dj e", j=8, dj=8)
        w2t3 = w2t[:].rearrange("p (j c) -> p j c", j=8)
        w2src = w2t_d.rearrange("(ej p) c -> p ej c", p=128)

        # sync ring (HW DGE): W1 pieces first, then x1.., then W2
        for jj in range(4):
            nc.sync.dma_start(w1t[:, 2048 * jj:2048 * (jj + 1)],
                              w1t_d[:, 2048 * jj:2048 * (jj + 1)])
        nc.sync.dma_start(xt[:, CH_OFF[1]:CH_OFF[2]], xt_d[:, CH_OFF[1]:CH_OFF[2]])
        nc.sync.dma_start(xt[:, CH_OFF[2]:16384], xt_d[:, CH_OFF[2]:16384])
        nc.sync.dma_start(w2t3[:, 0:4, 0:C], w2src[:, 0:4, :])
        nc.sync.dma_start(w2t3[:, 4:8, 0:C], w2src[:, 4:8, :])
        # scalar ring: x0 first, then the tiny b1/b2
        nc.scalar.dma_start(xt[:, CH_OFF[0]:CH_OFF[1]], xt_d[:, CH_OFF[0]:CH_OFF[1]])
        nc.scalar.dma_start(b1_sb[:], b1_d[:])
        nc.scalar.dma_start(b2_sb[:], b2_d.rearrange("(a c) -> a c", a=1))

        # ---- PE warm-up junk (memsets first on vector so warms start ~t0) --
        junk_w = small.tile([128, 128], FP8)
        nc.vector.memset(junk_w[:], 0.0)
        junk_x = small.tile([128, 512], FP8)
        nc.vector.memset(junk_x[:], 0.0)
        msumA = ps_ms.tile([128, 512], FP32, name="msumA")
        msumB = ps_ms.tile([128, 512], FP32, name="msumB")
        for i in range(N_WARM):
            nc.tensor.matmul(msumA[:], junk_w[:], junk_x[:], start=True, stop=True)

        # ---- scaled identities for the spike-sum matmuls ----
        io = small.tile([128, 128], mybir.dt.int32)
        nc.gpsimd.iota(io[:], pattern=[[1, 128]], base=0, channel_multiplier=-1)
        identM = small.tile([128, 128], BF16)   # -1 on the diagonal
        identS = small.tile([128, 128], BF16)   # +0.5 on the diagonal
        nc.vector.tensor_scalar(identM[:], io[:], 0, None, op0=OP.is_equal)
        nc.vector.tensor_scalar(identS[:], io[:], 0, None, op0=OP.is_equal)
        nc.vector.tensor_scalar_mul(identM[:], identM[:], -1.0)
        nc.vector.tensor_scalar_mul(identS[:], identS[:], 0.5)
        id8 = small.tile([8, 8], FP32)          # for the b1 PE transpose
        nc.vector.tensor_scalar(id8[:], io[0:8, 0:8], 0, None, op0=OP.is_equal)
        negone = small.tile([128, 1], FP32)
        nc.vector.memset(negone[:], -1.0)
        ones = small.tile([1, BC], BF16)
        nc.vector.memset(ones[:], 1.0)

        # w state history: state 0 + 16 pair tiles [w_{2i+1}, w_{2i+2}].
        # Pair tiles make the cross-engine mask reads whole-tile (the tile
        # dep tracker mishandles sub-range reads of one big tile).
        w0 = small.tile([128, 512], BF16)
        nc.vector.memset(w0[:], 0.0)
        wpair = [persist.tile([128, 1024], BF16, name=f"wp{i}")
                 for i in range(16)]

        def wview(s):
            if s == 0:
                return w0[:]
            i, hl = (s - 1) // 2, (s - 1) % 2
            return wpair[i][:, hl * 512:(hl + 1) * 512]

        b1h = small.tile([128, 8], FP32)

        # ---- matmul1: h[e, tb] = W1 @ x.T, fused 0.5*h + 0.5*b1 into scan
        # layout. h_sb free index = t*512 + j*64 + b ----
        h_sb = persist.tile([128, T * 512], BF16)
        h3 = h_sb[:].rearrange("p (t x) -> p t x", x=512)

        ps_tiles = {}

        def mm1_mm(g, js=range(8)):
            t0, nt = CHUNKS[g]
            nf = nt * 64
            xtg = xt[:, CH_OFF[g]:CH_OFF[g] + 8 * nf].rearrange(
                "p (dj t) -> p dj t", dj=8)
            for j in js:
                ps = ps_h.tile([128, 512], FP32, tag="ps_h", name=f"psh_{g}_{j}")
                ps_tiles[(g, j)] = ps
                for dp in range(4):   # pairs of contraction tiles (DoubleRow)
                    nc.tensor.matmul(
                        ps[:, 0:nf],
                        w1t3[:, j, 2 * dp:2 * dp + 2, :],
                        xtg[:, 2 * dp:2 * dp + 2, :],
                        start=(dp == 0), stop=(dp == 3),
                        perf_mode=mybir.MatmulPerfMode.DoubleRow,
                    )

        def mm1_copy(g, engine=None, js=range(8)):
            t0, nt = CHUNKS[g]
            for j in js:
                ps = ps_tiles[(g, j)]
                if engine == "vector":
                    nc.vector.tensor_scalar(
                        h3[:, t0:t0 + nt, j * 64:(j + 1) * 64],
                        ps[:, 0:nt * 64].rearrange("p (t b) -> p t b", t=nt),
                        0.5 / W1_PRESCALE, b1h[:, j:j + 1],
                        op0=OP.mult, op1=OP.add)
                else:
                    nc.scalar.activation(
                        h3[:, t0:t0 + nt, j * 64:(j + 1) * 64],
                        ps[:, 0:nt * 64].rearrange("p (t b) -> p t b", t=nt),
                        AF.Identity, scale=0.5 / W1_PRESCALE,
                        bias=b1h[:, j:j + 1])

        # ---- LIF scan pieces ----
        lif = _lif_op()
        m_tiles = {}

        def lif_step(t):
            nc.vector._custom_dve(lif, out=wview(t + 1), in0=wview(t),
                                  in1=h3[:, t, :], s0=1.0, s1=0.5)

        def mask_lt(t):
            # inline A-half mask on VectorE: m = (w_{t+1} < 1), bf16, 2x mode
            m = mpool.tile([128, 512], BF16, tag="m", name=f"m{t}")
            m_tiles[t] = m
            nc.vector.tensor_scalar(m[:], wview(t + 1), 1.0, None,
                                    op0=OP.is_lt)

        def ident_mm(t, tgt, start, stop):
            idv = identS if 16 <= t < 24 else identM
            nc.tensor.matmul(tgt[:], idv[:], m_tiles[t][:],
                             start=start, stop=stop)

        # ---- emission ----
        # b1 PE-transpose into a spare msumA corner, then the 0.5x copy
        nc.tensor.transpose(msumA[:, 0:8], b1_sb[:], id8[:])
        nc.scalar.activation(b1h[:], msumA[:, 0:8], AF.Copy, scale=0.5)
        mm1_mm(0)
        mm1_copy(0, "scalar", [0, 2, 4, 6])
        mm1_copy(0, "vector", [1, 3, 5, 7])
        mm1_mm(1)
        mm1_copy(1)
        mm1_mm(2)

        # vector scan chain: A-half with inline masks
        for t in range(0, 16):
            lif_step(t)
            mask_lt(t)

        mm1_copy(2)
        # PE: A-half spike sums interleaved into mm1 so the PE queue's tail
        # after g4 is short (masks arrive at scan pace, well ahead)
        for t in range(0, 4):
            ident_mm(t, msumA, start=(t == 0), stop=False)
        mm1_mm(3)
        mm1_copy(3)
        for t in range(4, 12):
            ident_mm(t, msumA, start=False, stop=False)
        mm1_mm(4)
        mm1_copy(4)
        for t in range(12, 16):
            ident_mm(t, msumA, start=False, stop=(t == 15))
        # paced junk keeps the PE clock up while mm2a waits for ssumA
        # (own psum tile: msumA is being read, msumB is mid-accumulation)
        psj = ps_h.tile([128, 512], FP32, tag="ps_h", name="psj")
        for _ in range(5):
            nc.tensor.matmul(psj[:], junk_w[:], junk_x[:], start=True,
                             stop=True)

        # scalar: ssumA cast right after the copies (bias 16 -> exact fp8)
        ssumA = small.tile([128, 512], FP8)
        nc.scalar.activation(ssumA[:], msumA[:], AF.Copy, scale=1.0, bias=16.0)
        ssumA3 = ssumA[:].rearrange("p (j b) -> p j b", j=8)

        # vector: B-half pure LIF chain. Scalar sign-pair masks for t16..23
        # are emitted right after the LIF step that completes their pair
        # tile (emission order defines the write->read dependency).
        for t in range(16, 32):
            lif_step(t)
            if t % 2 == 1 and 17 <= t <= 23:
                k = (t - 17) // 2
                sp = small.tile([128, 1024], BF16, name=f"sp{k}")
                m_tiles[t - 1] = sp[:, 0:512]
                m_tiles[t] = sp[:, 512:1024]
                nc.scalar.activation(sp[:], wpair[8 + k][:], AF.Sign,
                                     bias=negone[:, 0:1])
        # vector post-scan: t24..31 masks as four 1024-wide pair ops
        mq = small.tile([128, 4096], BF16)
        for k in range(4):
            nc.vector.tensor_scalar(mq[:, k * 1024:(k + 1) * 1024],
                                    wpair[12 + k][:], 1.0, None, op0=OP.is_lt)
        for t in range(24, 32):
            m_tiles[t] = mq[:, (t - 24) * 512:(t - 23) * 512]

        # b2 row cast (emitted here, after the copy chain + signs, so its
        # ~1us one-partition op doesn't delay the h-copy stream)
        b2_32 = small.tile([1, C], BF16)
        nc.scalar.activation(b2_32[:], b2_sb[:], AF.Copy, scale=1.0)

        y_sb = small.tile([BC, 1024], FP32)
        psy = [ps_y.tile([BC, 512], FP32, tag="ps_y", name=f"psy{h}")
               for h in range(2)]

        def mm2(ssum3, first, last):
            for pj in range(4):   # DoubleRow pairs of e-tiles, loaded once
                for half in range(2):
                    n = 512 if half == 0 else C - 512
                    c0 = half * 512
                    nc.tensor.matmul(
                        psy[half][:, 0:n],
                        ssum3[:, 2 * pj:2 * pj + 2, :],
                        w2t3[:, 2 * pj:2 * pj + 2, c0:c0 + n],
                        start=(first and pj == 0),
                        stop=(last and pj == 3),
                        perf_mode=mybir.MatmulPerfMode.DoubleRow,
                    )
            if first:   # bias rides in the first accumulation pass only
                for half in range(2):
                    n = 512 if half == 0 else C - 512
                    c0 = half * 512
                    nc.tensor.matmul(psy[half][:, 0:n], ones[:],
                                     b2_32[:, c0:c0 + n], start=False, stop=False)

        mm2(ssumA3, True, False)   # runs while the scan tail continues

        # PE: B-half spike sums (masks stream from scalar signs + vector mq);
        # one junk after each sign pair's idents keeps the PE clock from
        # drooping while the next pair is still being produced
        for i, t in enumerate(range(16, 32)):
            ident_mm(t, msumB, start=(i == 0), stop=(i == 15))
            if t in (17, 19, 21):
                nc.tensor.matmul(psj[:], junk_w[:], junk_x[:], start=True,
                                 stop=True)

        ssumB = small.tile([128, 512], FP8)
        nc.scalar.activation(ssumB[:], msumB[:], AF.Copy, scale=1.0, bias=12.0)
        ssumB3 = ssumB[:].rearrange("p (j b) -> p j b", j=8)
        mm2(ssumB3, False, True)

        # ---- log_softmax over C. |y| <= ~35 so no max-shift needed; the
        # Exp's free-dim accumulator replaces the reduce. ----
        ez = small.tile([BC, 1024], BF16)
        esum = small.tile([BC, 2], FP32)
        esum1 = small.tile([BC, 1], FP32)
        lse = small.tile([BC, 1], FP32)
        out_sb = small.tile([BC, C], FP32)
        for half in range(2):
            n = 512 if half == 0 else C - 512
            c0 = half * 512
            # y_sb scaled copy on vector so the Exp (scalar) runs in parallel
            nc.vector.tensor_scalar_mul(y_sb[:, c0:c0 + n], psy[half][:, 0:n],
                                        1.0 / (T * W1_PRESCALE))
            nc.scalar.activation(ez[:, c0:c0 + n], y_sb[:, c0:c0 + n], AF.Exp,
                                 accum_out=esum[:, half:half + 1])
        nc.vector.tensor_tensor(esum1[:], esum[:, 0:1], esum[:, 1:2], op=OP.add)
        nc.scalar.activation(lse[:], esum1[:], AF.Ln)
        for half, ring in ((0, nc.sync), (1, nc.scalar)):
            n = 512 if half == 0 else C - 512
            c0 = half * 512
            nc.vector.tensor_scalar(out_sb[:, c0:c0 + n], y_sb[:, c0:c0 + n],
                                    lse[:], None, op0=OP.subtract)
            ring.dma_start(y_d[:, c0:c0 + n], out_sb[:, c0:c0 + n])

    nc.compile()
    return nc


_CACHE = {}


def kernel(x, W1, b1, W2, b2):
    if "nc" not in _CACHE:
        _CACHE["nc"] = build_program()
    nc = _CACHE["nc"]

    f8 = mybir.dt.np(FP8)
    x = np.asarray(x, dtype=np.float32)
    w1t = np.ascontiguousarray(
        (np.asarray(W1, dtype=np.float32).T * W1_PRESCALE).astype(f8)
        .reshape(128, 8, 8, 128)      # [p, dj, j, e']  (d = p*8+dj)
        .transpose(0, 2, 1, 3)        # -> [p, j, dj, e'] j-major pieces
    ).reshape(128, 8 * 1024)
    w2t = np.ascontiguousarray(
        (np.asarray(W2, dtype=np.float32).T * W1_PRESCALE).astype(f8))
    b1 = np.ascontiguousarray(
        np.asarray(b1, dtype=np.float32).reshape(8, 128))   # [j, p]
    b2_eff = np.ascontiguousarray(
        np.asarray(b2, dtype=np.float32) * (T * W1_PRESCALE))
    in_maps = []
    for i in range(N_CORES):
        xs = x[:, i * BC:(i + 1) * BC, :].reshape(TB, D).T.astype(f8)
        # [d, tb] -> per chunk [p, dj, nt*64] with d = p*8+dj, chunk-major
        xs4 = xs.reshape(128, 8, T, 64)
        blocks = [np.ascontiguousarray(
            xs4[:, :, t0:t0 + nt, :]).reshape(128, 8 * nt * 64)
            for (t0, nt) in CHUNKS]
        xhost = np.ascontiguousarray(np.concatenate(blocks, axis=1))
        in_maps.append({"xT": xhost, "W1T": w1t, "b1": b1, "W2T": w2t,
                        "b2": b2_eff})

    res = run_bass_kernel_spmd(nc, in_maps, core_ids=list(range(N_CORES)),
                               **_CACHE.get("run_kwargs", {}))
    _CACHE["last_results"] = res
    out = np.concatenate([res.results[i]["y"] for i in range(N_CORES)], axis=0)
    return out
